# revision 21
# baseline (speedup 1.0000x reference)
"""Trainium2 Bass kernel for nn_DecoderRNN (pointer-generator GRU decoder step).

Strategy (8 NeuronCores, SPMD, one NEFF):
  Phase A  - batch-parallel (16 rows/core): GRU cell, bilinear attention
             (encoder slice host-pre-transposed to [b][h, s]), context,
             pointer gate.
  AllGather - packed [16, 1425] row block (combined | p*attn | log(1-p)).
  Phase B  - vocab-parallel (6250 cols/core): logits = combined @ W_outT
             slice (bf16 weights, f32 PSUM), fused exp+row-sum, tiny
             AllReduce for the softmax denominator.
  Phase C  - pointer correction without any scatter: host-factorized lane
             matrices (K: s->lane one-hot incl. duplicate groups, BMT: lane
             batch mask, OH: lane->column one-hot) turn the data-dependent
             scatter into dense bf16 matmuls per chunk:
                 corr = ((K^T @ w^T) . BMT)^T @ OH
             merged output: out = log(exp(l + cb) + corr),
             cb = log(1-p) - log(sum_exp).
"""

import sys

sys.path.insert(0, "/opt/trn_rl_repo")

import numpy as np
import ml_dtypes

import concourse.bass as bass
import concourse.mybir as mybir
import concourse.tile as tile
from concourse import bacc
from concourse.alu_op_type import AluOpType
from concourse.bass_utils import run_bass_kernel_spmd
from concourse.masks import make_identity

B, S, E, H, V = 128, 400, 512, 512, 50000
NCORES = 8
BC = B // NCORES      # 16 batch rows per core
VC = V // NCORES      # 6250 vocab cols per core
H2 = 2 * H            # 1024
AGW = H2 + S + 1      # 1425 packed all-gather row width
F32 = mybir.dt.float32
BF16 = mybir.dt.bfloat16
I32 = mybir.dt.int32
AF = mybir.ActivationFunctionType

N_CHUNK = 512
CHUNKS = [(i * N_CHUNK, min(N_CHUNK, VC - i * N_CHUNK))
          for i in range((VC + N_CHUNK - 1) // N_CHUNK)]


# --------------------------------------------------------------------------
# host-side lane factorization of the pointer scatter
# --------------------------------------------------------------------------

def build_lane_plan(idx_bs, core):
    """Factorize this core's pointer scatter into lane matrices.

    A lane = one (b, v) hit in the core's vocab range, assigned to one of
    G_ch groups of 128 lanes within its 512-wide output chunk:
        M1T[l, b'] = sum_s K[s, l] * wT[s, b']   (K one-hot: lane l's s-set)
        UT[l, b']  = M1T[l, b'] * BMT[l, b']     (BMT: 1 only at b' == b_l)
        corr[b, n] += sum_l UT[l, b] * OH[l, n]  (OH: 1 at lane l's column)
    Returns Kmat [512, NL] (s-padded), BMT [NL, 128], OHm [NL, 512],
    chunk_groups (groups per chunk), NL.
    """
    v_lo = core * VC
    hits_by_chunk = [[] for _ in range(len(CHUNKS))]
    groups = {}
    for b_ in range(B):
        row = idx_bs[b_]
        for s_ in range(S):
            v_ = int(row[s_])
            if v_lo <= v_ < v_lo + VC:
                groups.setdefault((b_, v_), []).append(s_)
    for (b_, v_), ss in groups.items():
        ch = (v_ - v_lo) // N_CHUNK
        hits_by_chunk[ch].append((b_, v_ - v_lo - ch * N_CHUNK, ss))
    chunk_groups = [max(1, -(-len(h) // 128)) for h in hits_by_chunk]
    NL = 128 * sum(chunk_groups)
    Kmat = np.zeros((512, NL), np.float32)
    BMT = np.zeros((NL, 128), np.float32)
    OHm = np.zeros((NL, N_CHUNK), np.float32)
    lane = 0
    for ch, h in enumerate(hits_by_chunk):
        for gi in range(chunk_groups[ch]):
            blk = h[gi * 128:(gi + 1) * 128]
            for i, (b_, n_, ss) in enumerate(blk):
                l = lane + i
                for s_ in ss:
                    Kmat[s_, l] = 1.0
                BMT[l, b_] = 1.0
                OHm[l, n_] = 1.0
            lane += 128
    return (Kmat.astype(ml_dtypes.bfloat16), BMT.astype(ml_dtypes.bfloat16),
            OHm.astype(ml_dtypes.bfloat16), chunk_groups, NL)


# --------------------------------------------------------------------------
# device program
# --------------------------------------------------------------------------

def build_nc(NL, chunk_groups, b_ptr_val, variant=6, asub=9):
    nc = bacc.Bacc("TRN2", target_bir_lowering=False, debug=False,
                   num_devices=NCORES)

    # ---- external inputs (per-core contents differ, shapes identical)
    embT = nc.dram_tensor("embT", [E, BC], F32, kind="ExternalInput").ap()
    hidT = nc.dram_tensor("hidT", [H, BC], F32, kind="ExternalInput").ap()
    hid = nc.dram_tensor("hid", [BC, H], F32, kind="ExternalInput").ap()
    WihT = nc.dram_tensor("WihT", [E, 3 * H], F32, kind="ExternalInput").ap()
    WhhT = nc.dram_tensor("WhhT", [H, 3 * H], F32, kind="ExternalInput").ap()
    bih = nc.dram_tensor("bih", [1, 3 * H], F32, kind="ExternalInput").ap()
    bhh = nc.dram_tensor("bhh", [1, 3 * H], F32, kind="ExternalInput").ap()
    Wbil = nc.dram_tensor("Wbil", [H, H], F32, kind="ExternalInput").ap()
    WptrT = nc.dram_tensor("WptrT", [H2, 1], F32, kind="ExternalInput").ap()
    encT = nc.dram_tensor("encT", [BC, H, S], F32, kind="ExternalInput").ap()
    WoutT = nc.dram_tensor("WoutT", [H2, VC], BF16, kind="ExternalInput").ap()
    bout = nc.dram_tensor("bout", [1, VC], BF16, kind="ExternalInput").ap()
    Kmat = nc.dram_tensor("Kmat", [512, NL], BF16, kind="ExternalInput").ap()
    BMT = nc.dram_tensor("BMT", [NL, 128], BF16, kind="ExternalInput").ap()
    OHm = nc.dram_tensor("OHm", [NL, N_CHUNK], BF16, kind="ExternalInput").ap()

    # ---- external outputs
    out_flat = nc.dram_tensor("out_flat", [128 * VC], F32,
                              kind="ExternalOutput").ap()
    hnew_o = nc.dram_tensor("hnew_o", [BC, H], F32, kind="ExternalOutput").ap()
    attn_o = nc.dram_tensor("attn_o", [BC, S], F32, kind="ExternalOutput").ap()
    pptr_o = nc.dram_tensor("pptr_o", [BC, 1], F32, kind="ExternalOutput").ap()

    out_dense = out_flat.rearrange("(b v) -> b v", b=128)

    with tile.TileContext(nc) as tc:
        with (
            tc.tile_pool(name="const", bufs=1) as cpool,
            tc.tile_pool(name="phaseB", bufs=1) as bpool,
            tc.tile_pool(name="psum", bufs=1, space="PSUM") as pp,
            tc.tile_pool(name="dram", bufs=1, space="DRAM") as dpool,
        ):
            ident = cpool.tile([128, 128], F32)
            make_identity(nc, ident[:])
            ones_f = cpool.tile([1, 128], F32)
            nc.vector.memset(ones_f[:], 1.0)
            ones_b = cpool.tile([1, 128], BF16)
            nc.vector.memset(ones_b[:], 1.0)
            bptr_t = cpool.tile([BC, 1], F32)
            nc.vector.memset(bptr_t[:], float(b_ptr_val))
            hnew_sb = cpool.tile([BC, H], F32)
            hnT = cpool.tile([128, 4, BC], F32)

            # ================= phase A: GRU =================================
            with tc.tile_pool(name="gru", bufs=1) as gpool:
                embT_s = gpool.tile([128, 4, BC], F32)
                nc.sync.dma_start(out=embT_s[:], in_=embT.rearrange("(t p) b -> p t b", p=128))
                hidT_s = gpool.tile([128, 4, BC], F32)
                nc.sync.dma_start(out=hidT_s[:], in_=hidT.rearrange("(t p) b -> p t b", p=128))
                hid_s = gpool.tile([BC, H], F32)
                nc.sync.dma_start(out=hid_s[:], in_=hid)
                WihT_s = gpool.tile([128, 4, 3 * H], F32)
                nc.sync.dma_start(out=WihT_s[:], in_=WihT.rearrange("(t p) n -> p t n", p=128))
                WhhT_s = gpool.tile([128, 4, 3 * H], F32)
                nc.sync.dma_start(out=WhhT_s[:], in_=WhhT.rearrange("(t p) n -> p t n", p=128))
                bih_s = gpool.tile([1, 3 * H], F32)
                nc.sync.dma_start(out=bih_s[:], in_=bih)
                bhh_s = gpool.tile([1, 3 * H], F32)
                nc.sync.dma_start(out=bhh_s[:], in_=bhh)

                # gates, one 512-wide chunk (= one gate) at a time: r, z, n
                r_sb = gpool.tile([BC, H], F32)
                z_sb = gpool.tile([BC, H], F32)
                n_sb = gpool.tile([BC, H], F32)
                for nt, gate in ((0, r_sb), (1, z_sb), (2, n_sb)):
                    sl = slice(nt * 512, (nt + 1) * 512)
                    psum_gi = pp.tile([BC, 512], F32, tag="ps", bufs=6)
                    for kt in range(4):
                        nc.tensor.matmul(psum_gi[:], embT_s[:, kt, :], WihT_s[:, kt, sl],
                                         start=(kt == 0), stop=False)
                    nc.tensor.matmul(psum_gi[:], ones_f[0:1, :BC], bih_s[0:1, sl],
                                     start=False, stop=True)
                    psum_gh = pp.tile([BC, 512], F32, tag="ps", bufs=6)
                    for kt in range(4):
                        nc.tensor.matmul(psum_gh[:], hidT_s[:, kt, :], WhhT_s[:, kt, sl],
                                         start=(kt == 0), stop=False)
                    nc.tensor.matmul(psum_gh[:], ones_f[0:1, :BC], bhh_s[0:1, sl],
                                     start=False, stop=True)
                    pre = gpool.tile([BC, H], F32, tag="pre", bufs=2)
                    if nt < 2:
                        nc.vector.tensor_copy(pre[:], psum_gh[:])
                        nc.vector.tensor_add(pre[:], pre[:], psum_gi[:])
                        nc.scalar.activation(gate[:], pre[:], AF.Sigmoid)
                    else:
                        nc.vector.tensor_copy(pre[:], psum_gh[:])
                        nc.vector.tensor_mul(pre[:], r_sb[:], pre[:])
                        nc.vector.tensor_add(pre[:], pre[:], psum_gi[:])
                        nc.scalar.activation(gate[:], pre[:], AF.Tanh)
                d_sb = gpool.tile([BC, H], F32)
                nc.vector.tensor_sub(d_sb[:], hid_s[:], n_sb[:])
                nc.vector.tensor_mul(d_sb[:], z_sb[:], d_sb[:])
                nc.vector.tensor_add(hnew_sb[:], n_sb[:], d_sb[:])
                nc.sync.dma_start(out=hnew_o, in_=hnew_sb[:])

                # h_new^T  [128,(4),16]
                for ht in range(4):
                    ps_t = pp.tile([128, BC], F32, tag="ps", bufs=6)
                    nc.tensor.transpose(ps_t[:], hnew_sb[:, ht * 128:(ht + 1) * 128],
                                        ident[:BC, :BC])
                    nc.vector.tensor_copy(hnT[:, ht, :], ps_t[:])

            # ================= phase A: attention ===========================
            ag_in = dpool.tile([BC, AGW], F32)
            if variant >= 2:
                with tc.tile_pool(name="attn", bufs=1) as apool:
                    Wbil_s = apool.tile([128, 4, H], F32)
                    nc.sync.dma_start(out=Wbil_s[:], in_=Wbil.rearrange("(t p) n -> p t n", p=128))
                    WptrT_s = apool.tile([128, 8, 1], F32)
                    nc.sync.dma_start(out=WptrT_s[:], in_=WptrT.rearrange("(t p) o -> p t o", p=128))

                    # v = h_new @ W_bil ; v^T
                    psum_v = pp.tile([BC, H], F32, tag="ps", bufs=6)
                    for kt in range(4):
                        nc.tensor.matmul(psum_v[:], hnT[:, kt, :], Wbil_s[:, kt, :],
                                         start=(kt == 0), stop=(kt == 3))
                    v_sb = apool.tile([BC, H], F32)
                    nc.vector.tensor_copy(v_sb[:], psum_v[:])
                    vT = apool.tile([128, 4, BC], F32)
                    for ht in range(4):
                        ps_t2 = pp.tile([128, BC], F32, tag="ps", bufs=6)
                        nc.tensor.transpose(ps_t2[:], v_sb[:, ht * 128:(ht + 1) * 128],
                                            ident[:BC, :BC])
                        nc.vector.tensor_copy(vT[:, ht, :], ps_t2[:])

                    # per-row energy/softmax at partition 0; un-normalized ctx
                    exps = apool.tile([BC, S], F32)
                    se_row = apool.tile([1, BC], F32)
                    ctxT = apool.tile([128, 4, BC], F32)
                    for b_ in range(BC):
                        enc_b = apool.tile([128, 4, S], F32, tag="enc_b", bufs=3)
                        nc.sync.dma_start(out=enc_b[:], in_=encT[b_].rearrange("(t p) s -> p t s", p=128))
                        psum_e = pp.tile([1, S], F32, tag="ps", bufs=6)
                        for kt in range(4):
                            nc.tensor.matmul(psum_e[:], vT[:, kt, b_:b_ + 1], enc_b[:, kt, :],
                                             start=(kt == 0), stop=(kt == 3))
                        negmax_r = apool.tile([1, 1], F32, tag="negmax_r", bufs=2)
                        nc.vector.tensor_reduce(negmax_r[:], psum_e[:],
                                                axis=mybir.AxisListType.X, op=AluOpType.max,
                                                negate=True)
                        exp_row = apool.tile([1, S], F32, tag="exp_row", bufs=2)
                        if asub >= 2:
                            nc.scalar.activation(exp_row[:], psum_e[:], AF.Exp,
                                                 bias=negmax_r[:],
                                                 accum_out=se_row[0:1, b_:b_ + 1])
                        else:
                            nc.scalar.activation(exp_row[:], psum_e[:], AF.Exp,
                                                 bias=negmax_r[:])
                            nc.vector.tensor_reduce(se_row[0:1, b_:b_ + 1], exp_row[:],
                                                    axis=mybir.AxisListType.X,
                                                    op=AluOpType.add)
                        if asub >= 3:
                            nc.sync.dma_start(out=exps[b_:b_ + 1, :], in_=exp_row[:])
                        psum_bc = pp.tile([128, S], F32, tag="ps", bufs=6)
                        if asub >= 4:
                            nc.tensor.matmul(psum_bc[:], ones_f[0:1, :128], exp_row[:],
                                             start=True, stop=True)
                        if asub >= 5:
                            for kt in range(4):
                                scr = apool.tile([128, S], F32, tag="ttr_scr", bufs=2)
                                nc.vector.tensor_mul(scr[:], enc_b[:, kt, :], psum_bc[:])
                                nc.vector.tensor_reduce(ctxT[:, kt, b_:b_ + 1], scr[:],
                                                        axis=mybir.AxisListType.X,
                                                        op=AluOpType.add)
                        elif asub == 4:
                            scr0 = apool.tile([128, S], F32, tag="ttr_scr", bufs=2)
                            nc.vector.tensor_copy(scr0[:], psum_bc[:])
                    if asub < 6:
                        nc.vector.memset(ctxT[:], 0.001)
                        nc.vector.memset(se_row[:], 1.0)
                        if asub < 3:
                            nc.vector.memset(exps[:], 0.5)

                    # 1/rowsum as [16, 1]
                    ps_se = pp.tile([BC, 1], F32, tag="ps", bufs=6)
                    nc.tensor.transpose(ps_se[:], se_row[:], ident[:1, :1])
                    rec = apool.tile([BC, 1], F32)
                    nc.vector.reciprocal(rec[:], ps_se[:])
                    attn_sb = apool.tile([BC, S], F32)
                    nc.vector.tensor_scalar(out=attn_sb[:], in0=exps[:], scalar1=rec[:],
                                            scalar2=None, op0=AluOpType.mult)
                    nc.sync.dma_start(out=attn_o, in_=attn_sb[:])

                    # ctx -> [16, 512] (normalize by rec during PSUM->SBUF copy)
                    ctx_sb = apool.tile([BC, H], F32)
                    for ht in range(4):
                        ps_t3 = pp.tile([BC, 128], F32, tag="ps", bufs=6)
                        nc.tensor.transpose(ps_t3[:], ctxT[:, ht, :], ident[:128, :128])
                        nc.vector.tensor_scalar(out=ctx_sb[:, ht * 128:(ht + 1) * 128],
                                                in0=ps_t3[:], scalar1=rec[:], scalar2=None,
                                                op0=AluOpType.mult)

                    # pointer gate
                    psum_p1 = pp.tile([BC, 1], F32, tag="ps", bufs=6)
                    for kt in range(4):
                        nc.tensor.matmul(psum_p1[:], hnT[:, kt, :], WptrT_s[:, kt, :],
                                         start=(kt == 0), stop=(kt == 3))
                    psum_p2 = pp.tile([BC, 1], F32, tag="ps", bufs=6)
                    for kt in range(4):
                        nc.tensor.matmul(psum_p2[:], ctxT[:, kt, :], WptrT_s[:, 4 + kt, :],
                                         start=(kt == 0), stop=(kt == 3))
                    p1_sb = apool.tile([BC, 1], F32)
                    nc.vector.tensor_copy(p1_sb[:], psum_p1[:])
                    p_pre = apool.tile([BC, 1], F32)
                    nc.vector.tensor_scalar(out=p_pre[:], in0=psum_p2[:], scalar1=rec[:],
                                            scalar2=p1_sb[:], op0=AluOpType.mult,
                                            op1=AluOpType.add)
                    pptr_sb = apool.tile([BC, 1], F32)
                    nc.scalar.activation(pptr_sb[:], p_pre[:], AF.Sigmoid, bias=bptr_t[:])
                    nc.sync.dma_start(out=pptr_o, in_=pptr_sb[:])
                    log1mp = apool.tile([BC, 1], F32)
                    nc.scalar.activation(log1mp[:], pptr_sb[:], AF.Ln, bias=1.0, scale=-1.0)
                    w_sb = apool.tile([BC, S], F32)
                    nc.vector.tensor_scalar(out=w_sb[:], in0=attn_sb[:], scalar1=pptr_sb[:],
                                            scalar2=None, op0=AluOpType.mult)

                    # pack the all-gather block
                    nc.sync.dma_start(out=ag_in[:, 0:H], in_=hnew_sb[:])
                    nc.sync.dma_start(out=ag_in[:, H:H2], in_=ctx_sb[:])
                    nc.sync.dma_start(out=ag_in[:, H2:H2 + S], in_=w_sb[:])
                    nc.sync.dma_start(out=ag_in[:, H2 + S:AGW], in_=log1mp[:])

            # ================= AllGather ====================================
            if variant >= 3:
                ag_out = dpool.tile([B, AGW], F32, addr_space="Shared")
                nc.gpsimd.collective_compute(
                    "AllGather", AluOpType.bypass,
                    replica_groups=[list(range(NCORES))],
                    ins=[ag_in[:].opt()], outs=[ag_out[:].opt()])

                comb_full = bpool.tile([B, H2], F32)
                nc.sync.dma_start(out=comb_full[:], in_=ag_out[:, 0:H2])
                log1mp_full = bpool.tile([B, 1], F32)
                nc.sync.dma_start(out=log1mp_full[:], in_=ag_out[:, H2 + S:AGW])
                w_full = bpool.tile([B, S], F32)
                nc.sync.dma_start(out=w_full[:], in_=ag_out[:, H2:H2 + S])

                # w^T in bf16, s-padded to 512 (pad rows zeroed for K matmul)
                wT_bf = bpool.tile([128, 4, 128], BF16)
                nc.vector.memset(wT_bf[:], 0.0)
                for st in range(4):
                    ssz = min(128, S - st * 128)
                    ps_w = pp.tile([128, 128], F32, tag="ps", bufs=6)
                    nc.tensor.transpose(ps_w[:ssz, :], w_full[:, st * 128:st * 128 + ssz],
                                        ident[:, :])
                    nc.vector.tensor_copy(wT_bf[:ssz, st, :], ps_w[:ssz, :])

            # ================= logits / exp-sum / corr ======================
            if variant >= 4:
                combT = bpool.tile([128, 8, 128], BF16)
                for kt in range(8):
                    ps_tr = pp.tile([128, 128], F32, tag="ps", bufs=6)
                    nc.tensor.transpose(ps_tr[:], comb_full[:, kt * 128:(kt + 1) * 128],
                                        ident[:, :])
                    nc.vector.tensor_copy(combT[:, kt, :], ps_tr[:])

                l_sb = bpool.tile([128, VC], F32)
                corr_sb = bpool.tile([128, VC], BF16)
                se_acc = bpool.tile([128, 1], F32)
                nc.vector.memset(se_acc[:], 0.0)
                lane0 = [0]
                for ci in range(len(CHUNKS)):
                    lane0.append(lane0[ci] + 128 * chunk_groups[ci])
                for ci, (n0, nch) in enumerate(CHUNKS):
                    wt = bpool.tile([128, 8, N_CHUNK], BF16, tag="wt", bufs=3)
                    nc.scalar.dma_start(out=wt[:, :, :nch],
                                        in_=WoutT[:, n0:n0 + nch].rearrange("(t p) n -> p t n", p=128))
                    bout_c = bpool.tile([1, N_CHUNK], BF16, tag="bout_c", bufs=2)
                    nc.scalar.dma_start(out=bout_c[:, :nch], in_=bout[0:1, n0:n0 + nch])
                    psum_l = pp.tile([128, N_CHUNK], F32, tag="ps", bufs=6)
                    for kt in range(8):
                        nc.tensor.matmul(psum_l[:, :nch], combT[:, kt, :], wt[:, kt, :nch],
                                         start=(kt == 0), stop=False)
                    nc.tensor.matmul(psum_l[:, :nch], ones_b[0:1, :128], bout_c[0:1, :nch],
                                     start=False, stop=True)
                    exp_scr = bpool.tile([128, N_CHUNK], F32, tag="exp_scr", bufs=2)
                    se_part = bpool.tile([128, 1], F32, tag="se_part", bufs=2)
                    nc.scalar.activation(exp_scr[:, :nch], psum_l[:, :nch], AF.Exp,
                                         accum_out=se_part[:])
                    nc.vector.tensor_add(se_acc[:], se_acc[:], se_part[:])
                    nc.vector.tensor_copy(l_sb[:, n0:n0 + nch], psum_l[:, :nch])

                    # pointer correction for this chunk
                    G = chunk_groups[ci]
                    lo = lane0[ci]
                    K_ch = bpool.tile([128, 4, 128 * G], BF16, tag="K_ch", bufs=2)
                    nc.scalar.dma_start(
                        out=K_ch[:],
                        in_=Kmat[:, lo:lo + 128 * G].rearrange("(t p) l -> p t l", p=128))
                    BMT_ch = bpool.tile([128, G, 128], BF16, tag="BMT_ch", bufs=2)
                    nc.scalar.dma_start(
                        out=BMT_ch[:],
                        in_=BMT[lo:lo + 128 * G, :].rearrange("(g p) b -> p g b", p=128))
                    OH_ch = bpool.tile([128, G, N_CHUNK], BF16, tag="OH_ch", bufs=2)
                    nc.scalar.dma_start(
                        out=OH_ch[:],
                        in_=OHm[lo:lo + 128 * G, :].rearrange("(g p) n -> p g n", p=128))
                    psum_corr = pp.tile([128, N_CHUNK], F32, tag="ps", bufs=6)
                    for g in range(G):
                        psum_m1 = pp.tile([128, 128], F32, tag="ps", bufs=6)
                        for kt in range(4):
                            nc.tensor.matmul(psum_m1[:], K_ch[:, kt, g * 128:(g + 1) * 128],
                                             wT_bf[:, kt, :], start=(kt == 0), stop=(kt == 3))
                        UT = bpool.tile([128, 128], BF16, tag="UT", bufs=3)
                        nc.vector.tensor_mul(UT[:], psum_m1[:], BMT_ch[:, g, :])
                        nc.tensor.matmul(psum_corr[:, :nch], UT[:], OH_ch[:, g, :nch],
                                         start=(g == 0), stop=(g == G - 1))
                    nc.vector.tensor_copy(corr_sb[:, n0:n0 + nch], psum_corr[:, :nch])

            # ================= AllReduce of softmax denominator =============
            if variant >= 5:
                ar_in = dpool.tile([128, 1], F32)
                nc.sync.dma_start(out=ar_in[:], in_=se_acc[:])
                ar_out = dpool.tile([128, 1], F32, addr_space="Shared")
                nc.gpsimd.collective_compute(
                    "AllReduce", AluOpType.add,
                    replica_groups=[list(range(NCORES))],
                    ins=[ar_in[:].opt()], outs=[ar_out[:].opt()])
                Ssum = bpool.tile([128, 1], F32)
                nc.sync.dma_start(out=Ssum[:], in_=ar_out[:])
                lnS = bpool.tile([128, 1], F32)
                nc.scalar.activation(lnS[:], Ssum[:], AF.Ln)
                cb_s = bpool.tile([128, 1], F32)
                nc.vector.tensor_sub(cb_s[:], log1mp_full[:], lnS[:])

            # ================= merged output ================================
            if variant >= 6:
                # out = log(exp(l + cb) + corr), chunk by chunk
                for n0, nch in CHUNKS:
                    e_t = bpool.tile([128, N_CHUNK], F32, tag="e_t", bufs=2)
                    nc.scalar.activation(e_t[:, :nch], l_sb[:, n0:n0 + nch], AF.Exp,
                                         bias=cb_s[:])
                    nc.vector.tensor_add(e_t[:, :nch], e_t[:, :nch],
                                         corr_sb[:, n0:n0 + nch])
                    o_t = bpool.tile([128, N_CHUNK], F32, tag="o_t", bufs=2)
                    nc.scalar.activation(o_t[:, :nch], e_t[:, :nch], AF.Ln)
                    nc.sync.dma_start(out=out_dense[:, n0:n0 + nch], in_=o_t[:, :nch])

    nc.compile()
    return nc


# --------------------------------------------------------------------------
# host wrapper
# --------------------------------------------------------------------------

_NC_CACHE = {}
TRACE = False
LAST_RESULT = None
VARIANT = 6
ASUB = 9


def _get_nc(NL, chunk_groups, b_ptr_val):
    key = (NL, tuple(chunk_groups), float(b_ptr_val), VARIANT, ASUB)
    if key not in _NC_CACHE:
        _NC_CACHE[key] = build_nc(NL, tuple(chunk_groups), b_ptr_val, VARIANT, ASUB)
    return _NC_CACHE[key]


def make_in_maps(inputs):
    emb = np.ascontiguousarray(np.asarray(inputs["embedded"], dtype=np.float32))
    hidf = np.ascontiguousarray(np.asarray(inputs["hidden"], dtype=np.float32))[0]
    enc = np.asarray(inputs["encoder_states"], dtype=np.float32)
    idx = np.asarray(inputs["encoder_word_idx"])
    W_ih = np.asarray(inputs["W_ih"], dtype=np.float32)
    W_hh = np.asarray(inputs["W_hh"], dtype=np.float32)
    b_ih = np.asarray(inputs["b_ih"], dtype=np.float32)
    b_hh = np.asarray(inputs["b_hh"], dtype=np.float32)
    W_bil = np.asarray(inputs["W_bil"], dtype=np.float32)[0]
    W_out = np.asarray(inputs["W_out"], dtype=np.float32)
    b_out = np.asarray(inputs["b_out"], dtype=np.float32)
    W_ptr = np.asarray(inputs["W_ptr"], dtype=np.float32)
    b_ptr = float(np.asarray(inputs["b_ptr"], dtype=np.float32)[0])

    idx_bs = idx.T  # [B, S]

    WihT = np.ascontiguousarray(W_ih.T)
    WhhT = np.ascontiguousarray(W_hh.T)
    Wbil = np.ascontiguousarray(W_bil)
    WptrT = np.ascontiguousarray(W_ptr.T)
    bih = b_ih[None, :]
    bhh = b_hh[None, :]

    in_maps = []
    plans = []
    for c in range(NCORES):
        bs = slice(c * BC, (c + 1) * BC)
        vs = slice(c * VC, (c + 1) * VC)
        Km, Bm, Om, chunk_groups, NL = build_lane_plan(idx_bs, c)
        plans.append((chunk_groups, NL))
        in_maps.append({
            "embT": np.ascontiguousarray(emb[bs].T),
            "hidT": np.ascontiguousarray(hidf[bs].T),
            "hid": np.ascontiguousarray(hidf[bs]),
            "WihT": WihT, "WhhT": WhhT, "bih": bih, "bhh": bhh,
            "Wbil": Wbil, "WptrT": WptrT,
            "encT": np.ascontiguousarray(enc[:, bs, :].transpose(1, 2, 0)),
            "WoutT": np.ascontiguousarray(W_out[vs].T).astype(ml_dtypes.bfloat16),
            "bout": b_out[None, vs].astype(ml_dtypes.bfloat16),
            "Kmat": Km, "BMT": Bm, "OHm": Om,
        })
    # all cores must share one NEFF: unify lane-plan shapes to the max
    gmax = [max(p[0][ci] for p in plans) for ci in range(len(CHUNKS))]
    NLu = 128 * sum(gmax)
    for c in range(NCORES):
        chunk_groups, NL = plans[c]
        Kn = np.zeros((512, NLu), ml_dtypes.bfloat16)
        Bn = np.zeros((NLu, 128), ml_dtypes.bfloat16)
        On = np.zeros((NLu, N_CHUNK), ml_dtypes.bfloat16)
        src_lo = 0
        dst_lo = 0
        for ci in range(len(CHUNKS)):
            n = 128 * chunk_groups[ci]
            Kn[:, dst_lo:dst_lo + n] = in_maps[c]["Kmat"][:, src_lo:src_lo + n]
            Bn[dst_lo:dst_lo + n] = in_maps[c]["BMT"][src_lo:src_lo + n]
            On[dst_lo:dst_lo + n] = in_maps[c]["OHm"][src_lo:src_lo + n]
            src_lo += n
            dst_lo += 128 * gmax[ci]
        in_maps[c]["Kmat"] = Kn
        in_maps[c]["BMT"] = Bn
        in_maps[c]["OHm"] = On
    return in_maps, b_ptr, gmax, NLu


def kernel(**inputs):
    global LAST_RESULT
    in_maps, b_ptr, gmax, NLu = make_in_maps(inputs)
    nc = _get_nc(NLu, gmax, b_ptr)
    res = run_bass_kernel_spmd(nc, in_maps, core_ids=list(range(NCORES)),
                               trace=TRACE)
    LAST_RESULT = res
    results = res.results

    out = np.empty((B, V), np.float32)
    h_new = np.empty((B, H), np.float32)
    enc_attn = np.empty((B, S), np.float32)
    prob_ptr = np.empty((B, 1), np.float32)
    for c in range(NCORES):
        r = results[c]
        out[:, c * VC:(c + 1) * VC] = r["out_flat"].reshape(128, VC)
        h_new[c * BC:(c + 1) * BC] = r["hnew_o"]
        enc_attn[c * BC:(c + 1) * BC] = r["attn_o"]
        prob_ptr[c * BC:(c + 1) * BC] = r["pptr_o"]
    return out, h_new[None], enc_attn[:, :, None], prob_ptr


# revision 22
# speedup vs baseline: 1.0665x; 1.0665x over previous
"""Trainium2 Bass kernel for nn_DecoderRNN (pointer-generator GRU decoder step).

Strategy (8 NeuronCores, SPMD, one NEFF):
  Phase A  - batch-parallel (16 rows/core): GRU cell, bilinear attention
             (encoder slice host-pre-transposed to [b][h, s]), context,
             pointer gate.
  AllGather - packed [16, 1425] row block (combined | p*attn | log(1-p)).
  Phase B  - vocab-parallel (6250 cols/core): logits = combined @ W_outT
             slice (bf16 weights, f32 PSUM), fused exp+row-sum, tiny
             AllReduce for the softmax denominator.
  Phase C  - pointer correction without any scatter: host-factorized lane
             matrices (K: s->lane one-hot incl. duplicate groups, BMT: lane
             batch mask, OH: lane->column one-hot) turn the data-dependent
             scatter into dense bf16 matmuls per chunk:
                 corr = ((K^T @ w^T) . BMT)^T @ OH
             merged output: out = log(exp(l + cb) + corr),
             cb = log(1-p) - log(sum_exp).
"""

import sys

sys.path.insert(0, "/opt/trn_rl_repo")

import numpy as np
import ml_dtypes

import concourse.bass as bass
import concourse.mybir as mybir
import concourse.tile as tile
from concourse import bacc
from concourse.alu_op_type import AluOpType
from concourse.bass_utils import run_bass_kernel_spmd
from concourse.masks import make_identity

B, S, E, H, V = 128, 400, 512, 512, 50000
NCORES = 8
BC = B // NCORES      # 16 batch rows per core
VC = V // NCORES      # 6250 vocab cols per core
H2 = 2 * H            # 1024
AGW = H2 + S + 1      # 1425 packed all-gather row width
F32 = mybir.dt.float32
BF16 = mybir.dt.bfloat16
I32 = mybir.dt.int32
AF = mybir.ActivationFunctionType

N_CHUNK = 512
CHUNKS = [(i * N_CHUNK, min(N_CHUNK, VC - i * N_CHUNK))
          for i in range((VC + N_CHUNK - 1) // N_CHUNK)]


# --------------------------------------------------------------------------
# host-side lane factorization of the pointer scatter
# --------------------------------------------------------------------------

def build_lane_plan(idx_bs, core):
    """Factorize this core's pointer scatter into lane matrices.

    A lane = one (b, v) hit in the core's vocab range, assigned to one of
    G_ch groups of 128 lanes within its 512-wide output chunk:
        M1T[l, b'] = sum_s K[s, l] * wT[s, b']   (K one-hot: lane l's s-set)
        UT[l, b']  = M1T[l, b'] * BMT[l, b']     (BMT: 1 only at b' == b_l)
        corr[b, n] += sum_l UT[l, b] * OH[l, n]  (OH: 1 at lane l's column)
    Returns Kmat [512, NL] (s-padded), BMT [NL, 128], OHm [NL, 512],
    chunk_groups (groups per chunk), NL.
    """
    v_lo = core * VC
    hits_by_chunk = [[] for _ in range(len(CHUNKS))]
    groups = {}
    for b_ in range(B):
        row = idx_bs[b_]
        for s_ in range(S):
            v_ = int(row[s_])
            if v_lo <= v_ < v_lo + VC:
                groups.setdefault((b_, v_), []).append(s_)
    for (b_, v_), ss in groups.items():
        ch = (v_ - v_lo) // N_CHUNK
        hits_by_chunk[ch].append((b_, v_ - v_lo - ch * N_CHUNK, ss))
    chunk_groups = [max(1, -(-len(h) // 128)) for h in hits_by_chunk]
    NL = 128 * sum(chunk_groups)
    Kmat = np.zeros((512, NL), np.float32)
    BMT = np.zeros((NL, 128), np.float32)
    OHm = np.zeros((NL, N_CHUNK), np.float32)
    lane = 0
    for ch, h in enumerate(hits_by_chunk):
        for gi in range(chunk_groups[ch]):
            blk = h[gi * 128:(gi + 1) * 128]
            for i, (b_, n_, ss) in enumerate(blk):
                l = lane + i
                for s_ in ss:
                    Kmat[s_, l] = 1.0
                BMT[l, b_] = 1.0
                OHm[l, n_] = 1.0
            lane += 128
    return (Kmat.astype(ml_dtypes.bfloat16), BMT.astype(ml_dtypes.bfloat16),
            OHm.astype(ml_dtypes.bfloat16), chunk_groups, NL)


# --------------------------------------------------------------------------
# device program
# --------------------------------------------------------------------------

def build_nc(NL, chunk_groups, b_ptr_val, variant=6, asub=9):
    nc = bacc.Bacc("TRN2", target_bir_lowering=False, debug=False,
                   num_devices=NCORES)

    # ---- external inputs (per-core contents differ, shapes identical)
    embT = nc.dram_tensor("embT", [E, BC], F32, kind="ExternalInput").ap()
    hidT = nc.dram_tensor("hidT", [H, BC], F32, kind="ExternalInput").ap()
    hid = nc.dram_tensor("hid", [BC, H], F32, kind="ExternalInput").ap()
    WihT = nc.dram_tensor("WihT", [E, 3 * H], F32, kind="ExternalInput").ap()
    WhhT = nc.dram_tensor("WhhT", [H, 3 * H], F32, kind="ExternalInput").ap()
    bih = nc.dram_tensor("bih", [1, 3 * H], F32, kind="ExternalInput").ap()
    bhh = nc.dram_tensor("bhh", [1, 3 * H], F32, kind="ExternalInput").ap()
    Wbil = nc.dram_tensor("Wbil", [H, H], F32, kind="ExternalInput").ap()
    WptrT = nc.dram_tensor("WptrT", [H2, 1], F32, kind="ExternalInput").ap()
    encT = nc.dram_tensor("encT", [BC, H, S], F32, kind="ExternalInput").ap()
    WoutT = nc.dram_tensor("WoutT", [H2, VC], BF16, kind="ExternalInput").ap()
    bout = nc.dram_tensor("bout", [1, VC], BF16, kind="ExternalInput").ap()
    Kmat = nc.dram_tensor("Kmat", [512, NL], BF16, kind="ExternalInput").ap()
    BMT = nc.dram_tensor("BMT", [NL, 128], BF16, kind="ExternalInput").ap()
    OHm = nc.dram_tensor("OHm", [NL, N_CHUNK], BF16, kind="ExternalInput").ap()

    # ---- external outputs
    out_flat = nc.dram_tensor("out_flat", [128 * VC], F32,
                              kind="ExternalOutput").ap()
    hnew_o = nc.dram_tensor("hnew_o", [BC, H], F32, kind="ExternalOutput").ap()
    attn_o = nc.dram_tensor("attn_o", [BC, S], F32, kind="ExternalOutput").ap()
    pptr_o = nc.dram_tensor("pptr_o", [BC, 1], F32, kind="ExternalOutput").ap()

    out_dense = out_flat.rearrange("(b v) -> b v", b=128)

    with tile.TileContext(nc) as tc:
        with (
            tc.tile_pool(name="const", bufs=1) as cpool,
            tc.tile_pool(name="phaseB", bufs=1) as bpool,
            tc.tile_pool(name="psum", bufs=1, space="PSUM") as pp,
            tc.tile_pool(name="dram", bufs=1, space="DRAM") as dpool,
        ):
            ident = cpool.tile([128, 128], F32)
            make_identity(nc, ident[:])
            ones_f = cpool.tile([1, 128], F32)
            nc.vector.memset(ones_f[:], 1.0)
            ones_b = cpool.tile([1, 128], BF16)
            nc.vector.memset(ones_b[:], 1.0)
            bptr_t = cpool.tile([BC, 1], F32)
            nc.vector.memset(bptr_t[:], float(b_ptr_val))
            hnew_sb = cpool.tile([BC, H], F32)
            hnT = cpool.tile([128, 4, BC], F32)

            # ================= phase A: GRU =================================
            with tc.tile_pool(name="gru", bufs=1) as gpool:
                embT_s = gpool.tile([128, 4, BC], F32)
                nc.sync.dma_start(out=embT_s[:], in_=embT.rearrange("(t p) b -> p t b", p=128))
                hidT_s = gpool.tile([128, 4, BC], F32)
                nc.sync.dma_start(out=hidT_s[:], in_=hidT.rearrange("(t p) b -> p t b", p=128))
                hid_s = gpool.tile([BC, H], F32)
                nc.sync.dma_start(out=hid_s[:], in_=hid)
                WihT_s = gpool.tile([128, 4, 3 * H], F32)
                nc.sync.dma_start(out=WihT_s[:], in_=WihT.rearrange("(t p) n -> p t n", p=128))
                WhhT_s = gpool.tile([128, 4, 3 * H], F32)
                nc.sync.dma_start(out=WhhT_s[:], in_=WhhT.rearrange("(t p) n -> p t n", p=128))
                bih_s = gpool.tile([1, 3 * H], F32)
                nc.sync.dma_start(out=bih_s[:], in_=bih)
                bhh_s = gpool.tile([1, 3 * H], F32)
                nc.sync.dma_start(out=bhh_s[:], in_=bhh)

                # gates, one 512-wide chunk (= one gate) at a time: r, z, n
                r_sb = gpool.tile([BC, H], F32)
                z_sb = gpool.tile([BC, H], F32)
                n_sb = gpool.tile([BC, H], F32)
                for nt, gate in ((0, r_sb), (1, z_sb), (2, n_sb)):
                    sl = slice(nt * 512, (nt + 1) * 512)
                    psum_gi = pp.tile([BC, 512], F32, tag="ps", bufs=4)
                    for kt in range(4):
                        nc.tensor.matmul(psum_gi[:], embT_s[:, kt, :], WihT_s[:, kt, sl],
                                         start=(kt == 0), stop=False)
                    nc.tensor.matmul(psum_gi[:], ones_f[0:1, :BC], bih_s[0:1, sl],
                                     start=False, stop=True)
                    psum_gh = pp.tile([BC, 512], F32, tag="ps", bufs=4)
                    for kt in range(4):
                        nc.tensor.matmul(psum_gh[:], hidT_s[:, kt, :], WhhT_s[:, kt, sl],
                                         start=(kt == 0), stop=False)
                    nc.tensor.matmul(psum_gh[:], ones_f[0:1, :BC], bhh_s[0:1, sl],
                                     start=False, stop=True)
                    pre = gpool.tile([BC, H], F32, tag="pre", bufs=2)
                    if nt < 2:
                        nc.vector.tensor_copy(pre[:], psum_gh[:])
                        nc.vector.tensor_add(pre[:], pre[:], psum_gi[:])
                        nc.scalar.activation(gate[:], pre[:], AF.Sigmoid)
                    else:
                        nc.vector.tensor_copy(pre[:], psum_gh[:])
                        nc.vector.tensor_mul(pre[:], r_sb[:], pre[:])
                        nc.vector.tensor_add(pre[:], pre[:], psum_gi[:])
                        nc.scalar.activation(gate[:], pre[:], AF.Tanh)
                d_sb = gpool.tile([BC, H], F32)
                nc.vector.tensor_sub(d_sb[:], hid_s[:], n_sb[:])
                nc.vector.tensor_mul(d_sb[:], z_sb[:], d_sb[:])
                nc.vector.tensor_add(hnew_sb[:], n_sb[:], d_sb[:])
                nc.sync.dma_start(out=hnew_o, in_=hnew_sb[:])

                # h_new^T  [128,(4),16]
                for ht in range(4):
                    ps_t = pp.tile([128, BC], F32, tag="ps", bufs=4)
                    nc.tensor.transpose(ps_t[:], hnew_sb[:, ht * 128:(ht + 1) * 128],
                                        ident[:BC, :BC])
                    nc.vector.tensor_copy(hnT[:, ht, :], ps_t[:])

            # ================= phase A: attention ===========================
            ag_in = dpool.tile([BC, AGW], F32)
            if variant >= 2:
                with tc.tile_pool(name="attn", bufs=1) as apool:
                    Wbil_s = apool.tile([128, 4, H], F32)
                    nc.sync.dma_start(out=Wbil_s[:], in_=Wbil.rearrange("(t p) n -> p t n", p=128))
                    WptrT_s = apool.tile([128, 8, 1], F32)
                    nc.sync.dma_start(out=WptrT_s[:], in_=WptrT.rearrange("(t p) o -> p t o", p=128))

                    # v = h_new @ W_bil ; v^T
                    psum_v = pp.tile([BC, H], F32, tag="ps", bufs=4)
                    for kt in range(4):
                        nc.tensor.matmul(psum_v[:], hnT[:, kt, :], Wbil_s[:, kt, :],
                                         start=(kt == 0), stop=(kt == 3))
                    v_sb = apool.tile([BC, H], F32)
                    nc.vector.tensor_copy(v_sb[:], psum_v[:])
                    vT = apool.tile([128, 4, BC], F32)
                    for ht in range(4):
                        ps_t2 = pp.tile([128, BC], F32, tag="ps", bufs=4)
                        nc.tensor.transpose(ps_t2[:], v_sb[:, ht * 128:(ht + 1) * 128],
                                            ident[:BC, :BC])
                        nc.vector.tensor_copy(vT[:, ht, :], ps_t2[:])

                    # energy/softmax at partition 0, in two 8-row waves so
                    # independent rows pipeline; un-normalized context
                    exps = apool.tile([BC, S], F32)
                    se_row = apool.tile([1, BC], F32)
                    ctxT = apool.tile([128, 4, BC], F32)
                    for half in range(2):
                        enc_tiles = []
                        exp_tiles = []
                        for j in range(8):
                            b_ = half * 8 + j
                            enc_b = apool.tile([128, 4, S], F32, tag="enc_b", bufs=8)
                            nc.sync.dma_start(out=enc_b[:], in_=encT[b_].rearrange("(t p) s -> p t s", p=128))
                            enc_tiles.append(enc_b)
                            psum_e = pp.tile([1, S], F32, tag="ps_e", bufs=2)
                            for kt in range(4):
                                nc.tensor.matmul(psum_e[:], vT[:, kt, b_:b_ + 1], enc_b[:, kt, :],
                                                 start=(kt == 0), stop=(kt == 3))
                            negmax_r = apool.tile([1, 1], F32, tag="negmax_r", bufs=3)
                            nc.vector.tensor_reduce(negmax_r[:], psum_e[:],
                                                    axis=mybir.AxisListType.X, op=AluOpType.max,
                                                    negate=True)
                            exp_row = apool.tile([1, S], F32, tag="exp_row", bufs=8)
                            nc.scalar.activation(exp_row[:], psum_e[:], AF.Exp,
                                                 bias=negmax_r[:],
                                                 accum_out=se_row[0:1, b_:b_ + 1])
                            exp_tiles.append(exp_row)
                            nc.gpsimd.dma_start(out=exps[b_:b_ + 1, :], in_=exp_row[:])
                        for j in range(8):
                            b_ = half * 8 + j
                            enc_b = enc_tiles[j]
                            psum_bc = pp.tile([128, S], F32, tag="ps_bc", bufs=2)
                            nc.tensor.matmul(psum_bc[:], ones_f[0:1, :128], exp_tiles[j][:],
                                             start=True, stop=True)
                            for kt in range(4):
                                scr = apool.tile([128, S], F32, tag="ttr_scr", bufs=3)
                                nc.vector.tensor_mul(scr[:], enc_b[:, kt, :], psum_bc[:])
                                nc.vector.tensor_reduce(ctxT[:, kt, b_:b_ + 1], scr[:],
                                                        axis=mybir.AxisListType.X,
                                                        op=AluOpType.add)

                    # 1/rowsum as [16, 1]
                    ps_se = pp.tile([BC, 1], F32, tag="ps", bufs=4)
                    nc.tensor.transpose(ps_se[:], se_row[:], ident[:1, :1])
                    rec = apool.tile([BC, 1], F32)
                    nc.vector.reciprocal(rec[:], ps_se[:])
                    attn_sb = apool.tile([BC, S], F32)
                    nc.vector.tensor_scalar(out=attn_sb[:], in0=exps[:], scalar1=rec[:],
                                            scalar2=None, op0=AluOpType.mult)
                    nc.sync.dma_start(out=attn_o, in_=attn_sb[:])

                    # ctx -> [16, 512] (normalize by rec during PSUM->SBUF copy)
                    ctx_sb = apool.tile([BC, H], F32)
                    for ht in range(4):
                        ps_t3 = pp.tile([BC, 128], F32, tag="ps", bufs=4)
                        nc.tensor.transpose(ps_t3[:], ctxT[:, ht, :], ident[:128, :128])
                        nc.vector.tensor_scalar(out=ctx_sb[:, ht * 128:(ht + 1) * 128],
                                                in0=ps_t3[:], scalar1=rec[:], scalar2=None,
                                                op0=AluOpType.mult)

                    # pointer gate
                    psum_p1 = pp.tile([BC, 1], F32, tag="ps", bufs=4)
                    for kt in range(4):
                        nc.tensor.matmul(psum_p1[:], hnT[:, kt, :], WptrT_s[:, kt, :],
                                         start=(kt == 0), stop=(kt == 3))
                    psum_p2 = pp.tile([BC, 1], F32, tag="ps", bufs=4)
                    for kt in range(4):
                        nc.tensor.matmul(psum_p2[:], ctxT[:, kt, :], WptrT_s[:, 4 + kt, :],
                                         start=(kt == 0), stop=(kt == 3))
                    p1_sb = apool.tile([BC, 1], F32)
                    nc.vector.tensor_copy(p1_sb[:], psum_p1[:])
                    p_pre = apool.tile([BC, 1], F32)
                    nc.vector.tensor_scalar(out=p_pre[:], in0=psum_p2[:], scalar1=rec[:],
                                            scalar2=p1_sb[:], op0=AluOpType.mult,
                                            op1=AluOpType.add)
                    pptr_sb = apool.tile([BC, 1], F32)
                    nc.scalar.activation(pptr_sb[:], p_pre[:], AF.Sigmoid, bias=bptr_t[:])
                    nc.sync.dma_start(out=pptr_o, in_=pptr_sb[:])
                    log1mp = apool.tile([BC, 1], F32)
                    nc.scalar.activation(log1mp[:], pptr_sb[:], AF.Ln, bias=1.0, scale=-1.0)
                    w_sb = apool.tile([BC, S], F32)
                    nc.vector.tensor_scalar(out=w_sb[:], in0=attn_sb[:], scalar1=pptr_sb[:],
                                            scalar2=None, op0=AluOpType.mult)

                    # pack the all-gather block
                    nc.sync.dma_start(out=ag_in[:, 0:H], in_=hnew_sb[:])
                    nc.sync.dma_start(out=ag_in[:, H:H2], in_=ctx_sb[:])
                    nc.sync.dma_start(out=ag_in[:, H2:H2 + S], in_=w_sb[:])
                    nc.sync.dma_start(out=ag_in[:, H2 + S:AGW], in_=log1mp[:])

            # ================= AllGather ====================================
            if variant >= 3:
                ag_out = dpool.tile([B, AGW], F32, addr_space="Shared")
                nc.gpsimd.collective_compute(
                    "AllGather", AluOpType.bypass,
                    replica_groups=[list(range(NCORES))],
                    ins=[ag_in[:].opt()], outs=[ag_out[:].opt()])

                comb_full = bpool.tile([B, H2], F32)
                nc.sync.dma_start(out=comb_full[:], in_=ag_out[:, 0:H2])
                log1mp_full = bpool.tile([B, 1], F32)
                nc.sync.dma_start(out=log1mp_full[:], in_=ag_out[:, H2 + S:AGW])
                w_full = bpool.tile([B, S], F32)
                nc.sync.dma_start(out=w_full[:], in_=ag_out[:, H2:H2 + S])

                # w^T in bf16, s-padded to 512 (pad rows zeroed for K matmul)
                wT_bf = bpool.tile([128, 4, 128], BF16)
                nc.vector.memset(wT_bf[:], 0.0)
                for st in range(4):
                    ssz = min(128, S - st * 128)
                    ps_w = pp.tile([128, 128], F32, tag="ps", bufs=4)
                    nc.tensor.transpose(ps_w[:ssz, :], w_full[:, st * 128:st * 128 + ssz],
                                        ident[:, :])
                    nc.vector.tensor_copy(wT_bf[:ssz, st, :], ps_w[:ssz, :])

            # ================= logits / exp-sum / corr ======================
            if variant >= 4:
                combT = bpool.tile([128, 8, 128], BF16)
                for kt in range(8):
                    ps_tr = pp.tile([128, 128], F32, tag="ps", bufs=4)
                    nc.tensor.transpose(ps_tr[:], comb_full[:, kt * 128:(kt + 1) * 128],
                                        ident[:, :])
                    nc.vector.tensor_copy(combT[:, kt, :], ps_tr[:])

                elsb = bpool.tile([128, VC], F32)
                corr_sb = bpool.tile([128, VC], BF16)
                se_acc = bpool.tile([128, 1], F32)
                nc.vector.memset(se_acc[:], 0.0)
                lane0 = [0]
                for ci in range(len(CHUNKS)):
                    lane0.append(lane0[ci] + 128 * chunk_groups[ci])
                for ci, (n0, nch) in enumerate(CHUNKS):
                    wt = bpool.tile([128, 8, N_CHUNK], BF16, tag="wt", bufs=3)
                    nc.scalar.dma_start(out=wt[:, :, :nch],
                                        in_=WoutT[:, n0:n0 + nch].rearrange("(t p) n -> p t n", p=128))
                    bout_c = bpool.tile([1, N_CHUNK], BF16, tag="bout_c", bufs=2)
                    nc.scalar.dma_start(out=bout_c[:, :nch], in_=bout[0:1, n0:n0 + nch])
                    psum_l = pp.tile([128, N_CHUNK], F32, tag="ps_e", bufs=2)
                    for kt in range(8):
                        nc.tensor.matmul(psum_l[:, :nch], combT[:, kt, :], wt[:, kt, :nch],
                                         start=(kt == 0), stop=False)
                    nc.tensor.matmul(psum_l[:, :nch], ones_b[0:1, :128], bout_c[0:1, :nch],
                                     start=False, stop=True)
                    se_part = bpool.tile([128, 1], F32, tag="se_part", bufs=2)
                    nc.scalar.activation(elsb[:, n0:n0 + nch], psum_l[:, :nch], AF.Exp,
                                         accum_out=se_part[:])
                    nc.vector.tensor_add(se_acc[:], se_acc[:], se_part[:])

                    # pointer correction for this chunk
                    G = chunk_groups[ci]
                    lo = lane0[ci]
                    K_ch = bpool.tile([128, 4, 128 * G], BF16, tag="K_ch", bufs=2)
                    nc.scalar.dma_start(
                        out=K_ch[:],
                        in_=Kmat[:, lo:lo + 128 * G].rearrange("(t p) l -> p t l", p=128))
                    BMT_ch = bpool.tile([128, G, 128], BF16, tag="BMT_ch", bufs=2)
                    nc.scalar.dma_start(
                        out=BMT_ch[:],
                        in_=BMT[lo:lo + 128 * G, :].rearrange("(g p) b -> p g b", p=128))
                    OH_ch = bpool.tile([128, G, N_CHUNK], BF16, tag="OH_ch", bufs=2)
                    nc.scalar.dma_start(
                        out=OH_ch[:],
                        in_=OHm[lo:lo + 128 * G, :].rearrange("(g p) n -> p g n", p=128))
                    psum_corr = pp.tile([128, N_CHUNK], F32, tag="ps_bc", bufs=2)
                    for g in range(G):
                        psum_m1 = pp.tile([128, 128], F32, tag="ps", bufs=4)
                        for kt in range(4):
                            nc.tensor.matmul(psum_m1[:], K_ch[:, kt, g * 128:(g + 1) * 128],
                                             wT_bf[:, kt, :], start=(kt == 0), stop=(kt == 3))
                        UT = bpool.tile([128, 128], BF16, tag="UT", bufs=3)
                        nc.vector.tensor_mul(UT[:], psum_m1[:], BMT_ch[:, g, :])
                        nc.tensor.matmul(psum_corr[:, :nch], UT[:], OH_ch[:, g, :nch],
                                         start=(g == 0), stop=(g == G - 1))
                    nc.vector.tensor_copy(corr_sb[:, n0:n0 + nch], psum_corr[:, :nch])

            # ================= AllReduce of softmax denominator =============
            if variant >= 5:
                ar_in = dpool.tile([128, 1], F32)
                nc.sync.dma_start(out=ar_in[:], in_=se_acc[:])
                ar_out = dpool.tile([128, 1], F32, addr_space="Shared")
                nc.gpsimd.collective_compute(
                    "AllReduce", AluOpType.add,
                    replica_groups=[list(range(NCORES))],
                    ins=[ar_in[:].opt()], outs=[ar_out[:].opt()])
                Ssum = bpool.tile([128, 1], F32)
                nc.sync.dma_start(out=Ssum[:], in_=ar_out[:])
                # Cb = exp(log(1-p) - log(S)) = (1-p)/S, computed without logs
                r1mp = bpool.tile([128, 1], F32)
                nc.scalar.activation(r1mp[:], log1mp_full[:], AF.Exp)
                rS = bpool.tile([128, 1], F32)
                nc.vector.reciprocal(rS[:], Ssum[:])
                Cb = bpool.tile([128, 1], F32)
                nc.vector.tensor_mul(Cb[:], r1mp[:], rS[:])

            # ================= merged output ================================
            if variant >= 6:
                # out = log(Cb * exp(l) + corr), chunk by chunk (single Ln
                # table load; exp(l) was produced in the chunk loop)
                for n0, nch in CHUNKS:
                    m_t = bpool.tile([128, N_CHUNK], F32, tag="m_t", bufs=2)
                    nc.vector.tensor_scalar(out=m_t[:, :nch], in0=elsb[:, n0:n0 + nch],
                                            scalar1=Cb[:], scalar2=None,
                                            op0=AluOpType.mult)
                    nc.vector.tensor_add(m_t[:, :nch], m_t[:, :nch],
                                         corr_sb[:, n0:n0 + nch])
                    o_t = bpool.tile([128, N_CHUNK], F32, tag="o_t", bufs=2)
                    nc.scalar.activation(o_t[:, :nch], m_t[:, :nch], AF.Ln)
                    nc.sync.dma_start(out=out_dense[:, n0:n0 + nch], in_=o_t[:, :nch])

    nc.compile()
    return nc


# --------------------------------------------------------------------------
# host wrapper
# --------------------------------------------------------------------------

_NC_CACHE = {}
TRACE = False
LAST_RESULT = None
VARIANT = 6
ASUB = 9


def _get_nc(NL, chunk_groups, b_ptr_val):
    key = (NL, tuple(chunk_groups), float(b_ptr_val), VARIANT, ASUB)
    if key not in _NC_CACHE:
        _NC_CACHE[key] = build_nc(NL, tuple(chunk_groups), b_ptr_val, VARIANT, ASUB)
    return _NC_CACHE[key]


def make_in_maps(inputs):
    emb = np.ascontiguousarray(np.asarray(inputs["embedded"], dtype=np.float32))
    hidf = np.ascontiguousarray(np.asarray(inputs["hidden"], dtype=np.float32))[0]
    enc = np.asarray(inputs["encoder_states"], dtype=np.float32)
    idx = np.asarray(inputs["encoder_word_idx"])
    W_ih = np.asarray(inputs["W_ih"], dtype=np.float32)
    W_hh = np.asarray(inputs["W_hh"], dtype=np.float32)
    b_ih = np.asarray(inputs["b_ih"], dtype=np.float32)
    b_hh = np.asarray(inputs["b_hh"], dtype=np.float32)
    W_bil = np.asarray(inputs["W_bil"], dtype=np.float32)[0]
    W_out = np.asarray(inputs["W_out"], dtype=np.float32)
    b_out = np.asarray(inputs["b_out"], dtype=np.float32)
    W_ptr = np.asarray(inputs["W_ptr"], dtype=np.float32)
    b_ptr = float(np.asarray(inputs["b_ptr"], dtype=np.float32)[0])

    idx_bs = idx.T  # [B, S]

    WihT = np.ascontiguousarray(W_ih.T)
    WhhT = np.ascontiguousarray(W_hh.T)
    Wbil = np.ascontiguousarray(W_bil)
    WptrT = np.ascontiguousarray(W_ptr.T)
    bih = b_ih[None, :]
    bhh = b_hh[None, :]

    in_maps = []
    plans = []
    for c in range(NCORES):
        bs = slice(c * BC, (c + 1) * BC)
        vs = slice(c * VC, (c + 1) * VC)
        Km, Bm, Om, chunk_groups, NL = build_lane_plan(idx_bs, c)
        plans.append((chunk_groups, NL))
        in_maps.append({
            "embT": np.ascontiguousarray(emb[bs].T),
            "hidT": np.ascontiguousarray(hidf[bs].T),
            "hid": np.ascontiguousarray(hidf[bs]),
            "WihT": WihT, "WhhT": WhhT, "bih": bih, "bhh": bhh,
            "Wbil": Wbil, "WptrT": WptrT,
            "encT": np.ascontiguousarray(enc[:, bs, :].transpose(1, 2, 0)),
            "WoutT": np.ascontiguousarray(W_out[vs].T).astype(ml_dtypes.bfloat16),
            "bout": b_out[None, vs].astype(ml_dtypes.bfloat16),
            "Kmat": Km, "BMT": Bm, "OHm": Om,
        })
    # all cores must share one NEFF: unify lane-plan shapes to the max
    gmax = [max(p[0][ci] for p in plans) for ci in range(len(CHUNKS))]
    NLu = 128 * sum(gmax)
    for c in range(NCORES):
        chunk_groups, NL = plans[c]
        Kn = np.zeros((512, NLu), ml_dtypes.bfloat16)
        Bn = np.zeros((NLu, 128), ml_dtypes.bfloat16)
        On = np.zeros((NLu, N_CHUNK), ml_dtypes.bfloat16)
        src_lo = 0
        dst_lo = 0
        for ci in range(len(CHUNKS)):
            n = 128 * chunk_groups[ci]
            Kn[:, dst_lo:dst_lo + n] = in_maps[c]["Kmat"][:, src_lo:src_lo + n]
            Bn[dst_lo:dst_lo + n] = in_maps[c]["BMT"][src_lo:src_lo + n]
            On[dst_lo:dst_lo + n] = in_maps[c]["OHm"][src_lo:src_lo + n]
            src_lo += n
            dst_lo += 128 * gmax[ci]
        in_maps[c]["Kmat"] = Kn
        in_maps[c]["BMT"] = Bn
        in_maps[c]["OHm"] = On
    return in_maps, b_ptr, gmax, NLu


def kernel(**inputs):
    global LAST_RESULT
    in_maps, b_ptr, gmax, NLu = make_in_maps(inputs)
    nc = _get_nc(NLu, gmax, b_ptr)
    res = run_bass_kernel_spmd(nc, in_maps, core_ids=list(range(NCORES)),
                               trace=TRACE)
    LAST_RESULT = res
    results = res.results

    out = np.empty((B, V), np.float32)
    h_new = np.empty((B, H), np.float32)
    enc_attn = np.empty((B, S), np.float32)
    prob_ptr = np.empty((B, 1), np.float32)
    for c in range(NCORES):
        r = results[c]
        out[:, c * VC:(c + 1) * VC] = r["out_flat"].reshape(128, VC)
        h_new[c * BC:(c + 1) * BC] = r["hnew_o"]
        enc_attn[c * BC:(c + 1) * BC] = r["attn_o"]
        prob_ptr[c * BC:(c + 1) * BC] = r["pptr_o"]
    return out, h_new[None], enc_attn[:, :, None], prob_ptr


# revision 23
# speedup vs baseline: 1.0729x; 1.0060x over previous
"""Trainium2 Bass kernel for nn_DecoderRNN (pointer-generator GRU decoder step).

Strategy (8 NeuronCores, SPMD, one NEFF):
  Phase A  - batch-parallel (16 rows/core): GRU cell, bilinear attention
             (encoder slice host-pre-transposed to [b][h, s]), context,
             pointer gate.
  AllGather - packed [16, 1425] row block (combined | p*attn | log(1-p)).
  Phase B  - vocab-parallel (6250 cols/core): logits = combined @ W_outT
             slice (bf16 weights, f32 PSUM), fused exp+row-sum, tiny
             AllReduce for the softmax denominator.
  Phase C  - pointer correction without any scatter: host-factorized lane
             matrices (K: s->lane one-hot incl. duplicate groups, BMT: lane
             batch mask, OH: lane->column one-hot) turn the data-dependent
             scatter into dense bf16 matmuls per chunk:
                 corr = ((K^T @ w^T) . BMT)^T @ OH
             merged output: out = log(exp(l + cb) + corr),
             cb = log(1-p) - log(sum_exp).
"""

import sys

sys.path.insert(0, "/opt/trn_rl_repo")

import numpy as np
import ml_dtypes

import concourse.bass as bass
import concourse.mybir as mybir
import concourse.tile as tile
from concourse import bacc
from concourse.alu_op_type import AluOpType
from concourse.bass_utils import run_bass_kernel_spmd
from concourse.masks import make_identity

B, S, E, H, V = 128, 400, 512, 512, 50000
NCORES = 8
BC = B // NCORES      # 16 batch rows per core
VC = V // NCORES      # 6250 vocab cols per core
H2 = 2 * H            # 1024
AGW = H2 + S + 1      # 1425 packed all-gather row width
F32 = mybir.dt.float32
BF16 = mybir.dt.bfloat16
I32 = mybir.dt.int32
AF = mybir.ActivationFunctionType

N_CHUNK = 512
CHUNKS = [(i * N_CHUNK, min(N_CHUNK, VC - i * N_CHUNK))
          for i in range((VC + N_CHUNK - 1) // N_CHUNK)]


# --------------------------------------------------------------------------
# host-side lane factorization of the pointer scatter
# --------------------------------------------------------------------------

def build_lane_plan(idx_bs, core):
    """Factorize this core's pointer scatter into lane matrices.

    A lane = one (b, v) hit in the core's vocab range, assigned to one of
    G_ch groups of 128 lanes within its 512-wide output chunk:
        M1T[l, b'] = sum_s K[s, l] * wT[s, b']   (K one-hot: lane l's s-set)
        UT[l, b']  = M1T[l, b'] * BMT[l, b']     (BMT: 1 only at b' == b_l)
        corr[b, n] += sum_l UT[l, b] * OH[l, n]  (OH: 1 at lane l's column)
    Returns Kmat [512, NL] (s-padded), BMT [NL, 128], OHm [NL, 512],
    chunk_groups (groups per chunk), NL.
    """
    v_lo = core * VC
    hits_by_chunk = [[] for _ in range(len(CHUNKS))]
    groups = {}
    for b_ in range(B):
        row = idx_bs[b_]
        for s_ in range(S):
            v_ = int(row[s_])
            if v_lo <= v_ < v_lo + VC:
                groups.setdefault((b_, v_), []).append(s_)
    for (b_, v_), ss in groups.items():
        ch = (v_ - v_lo) // N_CHUNK
        hits_by_chunk[ch].append((b_, v_ - v_lo - ch * N_CHUNK, ss))
    chunk_groups = [max(1, -(-len(h) // 128)) for h in hits_by_chunk]
    NL = 128 * sum(chunk_groups)
    Kmat = np.zeros((512, NL), np.float32)
    BMT = np.zeros((NL, 128), np.float32)
    OHm = np.zeros((NL, N_CHUNK), np.float32)
    lane = 0
    for ch, h in enumerate(hits_by_chunk):
        for gi in range(chunk_groups[ch]):
            blk = h[gi * 128:(gi + 1) * 128]
            for i, (b_, n_, ss) in enumerate(blk):
                l = lane + i
                for s_ in ss:
                    Kmat[s_, l] = 1.0
                BMT[l, b_] = 1.0
                OHm[l, n_] = 1.0
            lane += 128
    return (Kmat.astype(ml_dtypes.bfloat16), BMT.astype(ml_dtypes.bfloat16),
            OHm.astype(ml_dtypes.bfloat16), chunk_groups, NL)


# --------------------------------------------------------------------------
# device program
# --------------------------------------------------------------------------

def build_nc(NL, chunk_groups, b_ptr_val, variant=6, asub=9):
    nc = bacc.Bacc("TRN2", target_bir_lowering=False, debug=False,
                   num_devices=NCORES)

    # ---- external inputs (per-core contents differ, shapes identical)
    embT = nc.dram_tensor("embT", [E, BC], F32, kind="ExternalInput").ap()
    hidT = nc.dram_tensor("hidT", [H, BC], F32, kind="ExternalInput").ap()
    hid = nc.dram_tensor("hid", [BC, H], F32, kind="ExternalInput").ap()
    WihT = nc.dram_tensor("WihT", [E, 3 * H], F32, kind="ExternalInput").ap()
    WhhT = nc.dram_tensor("WhhT", [H, 3 * H], F32, kind="ExternalInput").ap()
    bih = nc.dram_tensor("bih", [1, 3 * H], F32, kind="ExternalInput").ap()
    bhh = nc.dram_tensor("bhh", [1, 3 * H], F32, kind="ExternalInput").ap()
    Wbil = nc.dram_tensor("Wbil", [H, H], F32, kind="ExternalInput").ap()
    WptrT = nc.dram_tensor("WptrT", [H2, 1], F32, kind="ExternalInput").ap()
    encT = nc.dram_tensor("encT", [BC, H, S], F32, kind="ExternalInput").ap()
    WoutT = nc.dram_tensor("WoutT", [H2, VC], BF16, kind="ExternalInput").ap()
    bout = nc.dram_tensor("bout", [1, VC], BF16, kind="ExternalInput").ap()
    Kmat = nc.dram_tensor("Kmat", [512, NL], BF16, kind="ExternalInput").ap()
    BMT = nc.dram_tensor("BMT", [NL, 128], BF16, kind="ExternalInput").ap()
    OHm = nc.dram_tensor("OHm", [NL, N_CHUNK], BF16, kind="ExternalInput").ap()

    # ---- external outputs
    out_flat = nc.dram_tensor("out_flat", [128 * VC], F32,
                              kind="ExternalOutput").ap()
    hnew_o = nc.dram_tensor("hnew_o", [BC, H], F32, kind="ExternalOutput").ap()
    attn_o = nc.dram_tensor("attn_o", [BC, S], F32, kind="ExternalOutput").ap()
    pptr_o = nc.dram_tensor("pptr_o", [BC, 1], F32, kind="ExternalOutput").ap()

    out_dense = out_flat.rearrange("(b v) -> b v", b=128)

    with tile.TileContext(nc) as tc:
        with (
            tc.tile_pool(name="const", bufs=1) as cpool,
            tc.tile_pool(name="phaseB", bufs=1) as bpool,
            tc.tile_pool(name="psum", bufs=1, space="PSUM") as pp,
            tc.tile_pool(name="dram", bufs=1, space="DRAM") as dpool,
        ):
            ident = cpool.tile([128, 128], F32)
            make_identity(nc, ident[:])
            ones_f = cpool.tile([1, 128], F32)
            nc.vector.memset(ones_f[:], 1.0)
            ones_b = cpool.tile([1, 128], BF16)
            nc.vector.memset(ones_b[:], 1.0)
            bptr_t = cpool.tile([BC, 1], F32)
            nc.vector.memset(bptr_t[:], float(b_ptr_val))
            hnew_sb = cpool.tile([BC, H], F32)
            hnT = cpool.tile([128, 4, BC], F32)

            # ================= phase A: GRU =================================
            with tc.tile_pool(name="gru", bufs=1) as gpool:
                embT_s = gpool.tile([128, 4, BC], F32)
                nc.sync.dma_start(out=embT_s[:], in_=embT.rearrange("(t p) b -> p t b", p=128))
                hidT_s = gpool.tile([128, 4, BC], F32)
                nc.sync.dma_start(out=hidT_s[:], in_=hidT.rearrange("(t p) b -> p t b", p=128))
                hid_s = gpool.tile([BC, H], F32)
                nc.sync.dma_start(out=hid_s[:], in_=hid)
                WihT_s = gpool.tile([128, 4, 3 * H], F32)
                nc.sync.dma_start(out=WihT_s[:], in_=WihT.rearrange("(t p) n -> p t n", p=128))
                WhhT_s = gpool.tile([128, 4, 3 * H], F32)
                nc.sync.dma_start(out=WhhT_s[:], in_=WhhT.rearrange("(t p) n -> p t n", p=128))
                bih_s = gpool.tile([1, 3 * H], F32)
                nc.sync.dma_start(out=bih_s[:], in_=bih)
                bhh_s = gpool.tile([1, 3 * H], F32)
                nc.sync.dma_start(out=bhh_s[:], in_=bhh)

                # gates, one 512-wide chunk (= one gate) at a time: r, z, n
                r_sb = gpool.tile([BC, H], F32)
                z_sb = gpool.tile([BC, H], F32)
                n_sb = gpool.tile([BC, H], F32)
                for nt, gate in ((0, r_sb), (1, z_sb), (2, n_sb)):
                    sl = slice(nt * 512, (nt + 1) * 512)
                    psum_gi = pp.tile([BC, 512], F32, tag="ps", bufs=4)
                    for kt in range(4):
                        nc.tensor.matmul(psum_gi[:], embT_s[:, kt, :], WihT_s[:, kt, sl],
                                         start=(kt == 0), stop=False)
                    nc.tensor.matmul(psum_gi[:], ones_f[0:1, :BC], bih_s[0:1, sl],
                                     start=False, stop=True)
                    psum_gh = pp.tile([BC, 512], F32, tag="ps", bufs=4)
                    for kt in range(4):
                        nc.tensor.matmul(psum_gh[:], hidT_s[:, kt, :], WhhT_s[:, kt, sl],
                                         start=(kt == 0), stop=False)
                    nc.tensor.matmul(psum_gh[:], ones_f[0:1, :BC], bhh_s[0:1, sl],
                                     start=False, stop=True)
                    pre = gpool.tile([BC, H], F32, tag="pre", bufs=2)
                    if nt < 2:
                        nc.vector.tensor_copy(pre[:], psum_gh[:])
                        nc.vector.tensor_add(pre[:], pre[:], psum_gi[:])
                        nc.scalar.activation(gate[:], pre[:], AF.Sigmoid)
                    else:
                        nc.vector.tensor_copy(pre[:], psum_gh[:])
                        nc.vector.tensor_mul(pre[:], r_sb[:], pre[:])
                        nc.vector.tensor_add(pre[:], pre[:], psum_gi[:])
                        nc.scalar.activation(gate[:], pre[:], AF.Tanh)
                d_sb = gpool.tile([BC, H], F32)
                nc.vector.tensor_sub(d_sb[:], hid_s[:], n_sb[:])
                nc.vector.tensor_mul(d_sb[:], z_sb[:], d_sb[:])
                nc.vector.tensor_add(hnew_sb[:], n_sb[:], d_sb[:])
                nc.sync.dma_start(out=hnew_o, in_=hnew_sb[:])

                # h_new^T  [128,(4),16]
                for ht in range(4):
                    ps_t = pp.tile([128, BC], F32, tag="ps", bufs=4)
                    nc.tensor.transpose(ps_t[:], hnew_sb[:, ht * 128:(ht + 1) * 128],
                                        ident[:BC, :BC])
                    nc.vector.tensor_copy(hnT[:, ht, :], ps_t[:])

            # ================= phase A: attention ===========================
            ag_in = dpool.tile([BC, AGW], F32)
            if variant >= 2:
                with tc.tile_pool(name="attn", bufs=1) as apool:
                    Wbil_s = apool.tile([128, 4, H], F32)
                    nc.sync.dma_start(out=Wbil_s[:], in_=Wbil.rearrange("(t p) n -> p t n", p=128))
                    WptrT_s = apool.tile([128, 8, 1], F32)
                    nc.sync.dma_start(out=WptrT_s[:], in_=WptrT.rearrange("(t p) o -> p t o", p=128))

                    # v = h_new @ W_bil ; v^T
                    psum_v = pp.tile([BC, H], F32, tag="ps", bufs=4)
                    for kt in range(4):
                        nc.tensor.matmul(psum_v[:], hnT[:, kt, :], Wbil_s[:, kt, :],
                                         start=(kt == 0), stop=(kt == 3))
                    v_sb = apool.tile([BC, H], F32)
                    nc.vector.tensor_copy(v_sb[:], psum_v[:])
                    vT = apool.tile([128, 4, BC], F32)
                    for ht in range(4):
                        ps_t2 = pp.tile([128, BC], F32, tag="ps", bufs=4)
                        nc.tensor.transpose(ps_t2[:], v_sb[:, ht * 128:(ht + 1) * 128],
                                            ident[:BC, :BC])
                        nc.vector.tensor_copy(vT[:, ht, :], ps_t2[:])

                    # energy/softmax at partition 0, in two 8-row waves so
                    # independent rows pipeline; un-normalized context
                    exps = apool.tile([BC, S], F32)
                    se_row = apool.tile([1, BC], F32)
                    ctxT = apool.tile([128, 4, BC], F32)
                    for wv in range(4):
                        enc_tiles = []
                        exp_tiles = []
                        for j in range(4):
                            b_ = wv * 4 + j
                            enc_b = apool.tile([128, 4, S], F32, tag="enc_b", bufs=8)
                            nc.sync.dma_start(out=enc_b[:], in_=encT[b_].rearrange("(t p) s -> p t s", p=128))
                            enc_tiles.append(enc_b)
                            psum_e = pp.tile([1, S], F32, tag="ps_e", bufs=2)
                            for kt in range(4):
                                nc.tensor.matmul(psum_e[:], vT[:, kt, b_:b_ + 1], enc_b[:, kt, :],
                                                 start=(kt == 0), stop=(kt == 3))
                            negmax_r = apool.tile([1, 1], F32, tag="negmax_r", bufs=3)
                            nc.vector.tensor_reduce(negmax_r[:], psum_e[:],
                                                    axis=mybir.AxisListType.X, op=AluOpType.max,
                                                    negate=True)
                            exp_row = apool.tile([1, S], F32, tag="exp_row", bufs=8)
                            nc.scalar.activation(exp_row[:], psum_e[:], AF.Exp,
                                                 bias=negmax_r[:],
                                                 accum_out=se_row[0:1, b_:b_ + 1])
                            exp_tiles.append(exp_row)
                            nc.gpsimd.dma_start(out=exps[b_:b_ + 1, :], in_=exp_row[:])
                        for j in range(4):
                            b_ = wv * 4 + j
                            enc_b = enc_tiles[j]
                            psum_bc = pp.tile([128, S], F32, tag="ps_bc", bufs=2)
                            nc.tensor.matmul(psum_bc[:], ones_f[0:1, :128], exp_tiles[j][:],
                                             start=True, stop=True)
                            bc_sb = apool.tile([128, S], F32, tag="bc_sb", bufs=3)
                            nc.vector.tensor_copy(bc_sb[:], psum_bc[:])
                            for kt in range(4):
                                scr = apool.tile([128, S], F32, tag="ttr_scr", bufs=3)
                                nc.vector.tensor_mul(scr[:], enc_b[:, kt, :], bc_sb[:])
                                trash = apool.tile([128, S], F32, tag="trash", bufs=2)
                                nc.scalar.activation(trash[:], scr[:], AF.Identity,
                                                     accum_out=ctxT[:, kt, b_:b_ + 1])

                    # 1/rowsum as [16, 1]
                    ps_se = pp.tile([BC, 1], F32, tag="ps", bufs=4)
                    nc.tensor.transpose(ps_se[:], se_row[:], ident[:1, :1])
                    rec = apool.tile([BC, 1], F32)
                    nc.vector.reciprocal(rec[:], ps_se[:])
                    attn_sb = apool.tile([BC, S], F32)
                    nc.vector.tensor_scalar(out=attn_sb[:], in0=exps[:], scalar1=rec[:],
                                            scalar2=None, op0=AluOpType.mult)
                    nc.sync.dma_start(out=attn_o, in_=attn_sb[:])

                    # ctx -> [16, 512] (normalize by rec during PSUM->SBUF copy)
                    ctx_sb = apool.tile([BC, H], F32)
                    for ht in range(4):
                        ps_t3 = pp.tile([BC, 128], F32, tag="ps", bufs=4)
                        nc.tensor.transpose(ps_t3[:], ctxT[:, ht, :], ident[:128, :128])
                        nc.vector.tensor_scalar(out=ctx_sb[:, ht * 128:(ht + 1) * 128],
                                                in0=ps_t3[:], scalar1=rec[:], scalar2=None,
                                                op0=AluOpType.mult)

                    # pointer gate
                    psum_p1 = pp.tile([BC, 1], F32, tag="ps", bufs=4)
                    for kt in range(4):
                        nc.tensor.matmul(psum_p1[:], hnT[:, kt, :], WptrT_s[:, kt, :],
                                         start=(kt == 0), stop=(kt == 3))
                    psum_p2 = pp.tile([BC, 1], F32, tag="ps", bufs=4)
                    for kt in range(4):
                        nc.tensor.matmul(psum_p2[:], ctxT[:, kt, :], WptrT_s[:, 4 + kt, :],
                                         start=(kt == 0), stop=(kt == 3))
                    p1_sb = apool.tile([BC, 1], F32)
                    nc.vector.tensor_copy(p1_sb[:], psum_p1[:])
                    p_pre = apool.tile([BC, 1], F32)
                    nc.vector.tensor_scalar(out=p_pre[:], in0=psum_p2[:], scalar1=rec[:],
                                            scalar2=p1_sb[:], op0=AluOpType.mult,
                                            op1=AluOpType.add)
                    pptr_sb = apool.tile([BC, 1], F32)
                    nc.scalar.activation(pptr_sb[:], p_pre[:], AF.Sigmoid, bias=bptr_t[:])
                    nc.sync.dma_start(out=pptr_o, in_=pptr_sb[:])
                    log1mp = apool.tile([BC, 1], F32)
                    nc.scalar.activation(log1mp[:], pptr_sb[:], AF.Ln, bias=1.0, scale=-1.0)
                    w_sb = apool.tile([BC, S], F32)
                    nc.vector.tensor_scalar(out=w_sb[:], in0=attn_sb[:], scalar1=pptr_sb[:],
                                            scalar2=None, op0=AluOpType.mult)

                    # pack the all-gather block
                    nc.sync.dma_start(out=ag_in[:, 0:H], in_=hnew_sb[:])
                    nc.sync.dma_start(out=ag_in[:, H:H2], in_=ctx_sb[:])
                    nc.sync.dma_start(out=ag_in[:, H2:H2 + S], in_=w_sb[:])
                    nc.sync.dma_start(out=ag_in[:, H2 + S:AGW], in_=log1mp[:])

            # ================= AllGather ====================================
            if variant >= 3:
                ag_out = dpool.tile([B, AGW], F32, addr_space="Shared")
                nc.gpsimd.collective_compute(
                    "AllGather", AluOpType.bypass,
                    replica_groups=[list(range(NCORES))],
                    ins=[ag_in[:].opt()], outs=[ag_out[:].opt()])

                comb_full = bpool.tile([B, H2], F32)
                nc.sync.dma_start(out=comb_full[:], in_=ag_out[:, 0:H2])
                log1mp_full = bpool.tile([B, 1], F32)
                nc.sync.dma_start(out=log1mp_full[:], in_=ag_out[:, H2 + S:AGW])
                w_full = bpool.tile([B, S], F32)
                nc.sync.dma_start(out=w_full[:], in_=ag_out[:, H2:H2 + S])

                # w^T in bf16, s-padded to 512 (pad rows zeroed for K matmul)
                wT_bf = bpool.tile([128, 4, 128], BF16)
                nc.vector.memset(wT_bf[:], 0.0)
                for st in range(4):
                    ssz = min(128, S - st * 128)
                    ps_w = pp.tile([128, 128], F32, tag="ps", bufs=4)
                    nc.tensor.transpose(ps_w[:ssz, :], w_full[:, st * 128:st * 128 + ssz],
                                        ident[:, :])
                    nc.vector.tensor_copy(wT_bf[:ssz, st, :], ps_w[:ssz, :])

            # ================= logits / exp-sum / corr ======================
            if variant >= 4:
                combT = bpool.tile([128, 8, 128], BF16)
                for kt in range(8):
                    ps_tr = pp.tile([128, 128], F32, tag="ps", bufs=4)
                    nc.tensor.transpose(ps_tr[:], comb_full[:, kt * 128:(kt + 1) * 128],
                                        ident[:, :])
                    nc.vector.tensor_copy(combT[:, kt, :], ps_tr[:])

                elsb = bpool.tile([128, VC], F32)
                corr_sb = bpool.tile([128, VC], BF16)
                se_acc = bpool.tile([128, 1], F32)
                nc.vector.memset(se_acc[:], 0.0)
                lane0 = [0]
                for ci in range(len(CHUNKS)):
                    lane0.append(lane0[ci] + 128 * chunk_groups[ci])
                for ci, (n0, nch) in enumerate(CHUNKS):
                    wt = bpool.tile([128, 8, N_CHUNK], BF16, tag="wt", bufs=3)
                    nc.scalar.dma_start(out=wt[:, :, :nch],
                                        in_=WoutT[:, n0:n0 + nch].rearrange("(t p) n -> p t n", p=128))
                    bout_c = bpool.tile([1, N_CHUNK], BF16, tag="bout_c", bufs=2)
                    nc.scalar.dma_start(out=bout_c[:, :nch], in_=bout[0:1, n0:n0 + nch])
                    psum_l = pp.tile([128, N_CHUNK], F32, tag="ps_e", bufs=2)
                    for kt in range(8):
                        nc.tensor.matmul(psum_l[:, :nch], combT[:, kt, :], wt[:, kt, :nch],
                                         start=(kt == 0), stop=False)
                    nc.tensor.matmul(psum_l[:, :nch], ones_b[0:1, :128], bout_c[0:1, :nch],
                                     start=False, stop=True)
                    se_part = bpool.tile([128, 1], F32, tag="se_part", bufs=2)
                    nc.scalar.activation(elsb[:, n0:n0 + nch], psum_l[:, :nch], AF.Exp,
                                         accum_out=se_part[:])
                    nc.vector.tensor_add(se_acc[:], se_acc[:], se_part[:])

                    # pointer correction for this chunk
                    G = chunk_groups[ci]
                    lo = lane0[ci]
                    K_ch = bpool.tile([128, 4, 128 * G], BF16, tag="K_ch", bufs=2)
                    nc.scalar.dma_start(
                        out=K_ch[:],
                        in_=Kmat[:, lo:lo + 128 * G].rearrange("(t p) l -> p t l", p=128))
                    BMT_ch = bpool.tile([128, G, 128], BF16, tag="BMT_ch", bufs=2)
                    nc.scalar.dma_start(
                        out=BMT_ch[:],
                        in_=BMT[lo:lo + 128 * G, :].rearrange("(g p) b -> p g b", p=128))
                    OH_ch = bpool.tile([128, G, N_CHUNK], BF16, tag="OH_ch", bufs=2)
                    nc.scalar.dma_start(
                        out=OH_ch[:],
                        in_=OHm[lo:lo + 128 * G, :].rearrange("(g p) n -> p g n", p=128))
                    psum_corr = pp.tile([128, N_CHUNK], F32, tag="ps_bc", bufs=2)
                    for g in range(G):
                        psum_m1 = pp.tile([128, 128], F32, tag="ps", bufs=4)
                        for kt in range(4):
                            nc.tensor.matmul(psum_m1[:], K_ch[:, kt, g * 128:(g + 1) * 128],
                                             wT_bf[:, kt, :], start=(kt == 0), stop=(kt == 3))
                        UT = bpool.tile([128, 128], BF16, tag="UT", bufs=3)
                        nc.vector.tensor_mul(UT[:], psum_m1[:], BMT_ch[:, g, :])
                        nc.tensor.matmul(psum_corr[:, :nch], UT[:], OH_ch[:, g, :nch],
                                         start=(g == 0), stop=(g == G - 1))
                    nc.vector.tensor_copy(corr_sb[:, n0:n0 + nch], psum_corr[:, :nch])

            # ================= AllReduce of softmax denominator =============
            if variant >= 5:
                ar_in = dpool.tile([128, 1], F32)
                nc.sync.dma_start(out=ar_in[:], in_=se_acc[:])
                ar_out = dpool.tile([128, 1], F32, addr_space="Shared")
                nc.gpsimd.collective_compute(
                    "AllReduce", AluOpType.add,
                    replica_groups=[list(range(NCORES))],
                    ins=[ar_in[:].opt()], outs=[ar_out[:].opt()])
                Ssum = bpool.tile([128, 1], F32)
                nc.sync.dma_start(out=Ssum[:], in_=ar_out[:])
                # Cb = exp(log(1-p) - log(S)) = (1-p)/S, computed without logs
                r1mp = bpool.tile([128, 1], F32)
                nc.scalar.activation(r1mp[:], log1mp_full[:], AF.Exp)
                rS = bpool.tile([128, 1], F32)
                nc.vector.reciprocal(rS[:], Ssum[:])
                Cb = bpool.tile([128, 1], F32)
                nc.vector.tensor_mul(Cb[:], r1mp[:], rS[:])

            # ================= merged output ================================
            if variant >= 6:
                # out = log(Cb * exp(l) + corr), chunk by chunk (single Ln
                # table load; exp(l) was produced in the chunk loop)
                for n0, nch in CHUNKS:
                    m_t = bpool.tile([128, N_CHUNK], F32, tag="m_t", bufs=2)
                    nc.vector.tensor_scalar(out=m_t[:, :nch], in0=elsb[:, n0:n0 + nch],
                                            scalar1=Cb[:], scalar2=None,
                                            op0=AluOpType.mult)
                    nc.vector.tensor_add(m_t[:, :nch], m_t[:, :nch],
                                         corr_sb[:, n0:n0 + nch])
                    o_t = bpool.tile([128, N_CHUNK], F32, tag="o_t", bufs=2)
                    nc.scalar.activation(o_t[:, :nch], m_t[:, :nch], AF.Ln)
                    nc.sync.dma_start(out=out_dense[:, n0:n0 + nch], in_=o_t[:, :nch])

    nc.compile()
    return nc


# --------------------------------------------------------------------------
# host wrapper
# --------------------------------------------------------------------------

_NC_CACHE = {}
TRACE = False
LAST_RESULT = None
VARIANT = 6
ASUB = 9


def _get_nc(NL, chunk_groups, b_ptr_val):
    key = (NL, tuple(chunk_groups), float(b_ptr_val), VARIANT, ASUB)
    if key not in _NC_CACHE:
        _NC_CACHE[key] = build_nc(NL, tuple(chunk_groups), b_ptr_val, VARIANT, ASUB)
    return _NC_CACHE[key]


def make_in_maps(inputs):
    emb = np.ascontiguousarray(np.asarray(inputs["embedded"], dtype=np.float32))
    hidf = np.ascontiguousarray(np.asarray(inputs["hidden"], dtype=np.float32))[0]
    enc = np.asarray(inputs["encoder_states"], dtype=np.float32)
    idx = np.asarray(inputs["encoder_word_idx"])
    W_ih = np.asarray(inputs["W_ih"], dtype=np.float32)
    W_hh = np.asarray(inputs["W_hh"], dtype=np.float32)
    b_ih = np.asarray(inputs["b_ih"], dtype=np.float32)
    b_hh = np.asarray(inputs["b_hh"], dtype=np.float32)
    W_bil = np.asarray(inputs["W_bil"], dtype=np.float32)[0]
    W_out = np.asarray(inputs["W_out"], dtype=np.float32)
    b_out = np.asarray(inputs["b_out"], dtype=np.float32)
    W_ptr = np.asarray(inputs["W_ptr"], dtype=np.float32)
    b_ptr = float(np.asarray(inputs["b_ptr"], dtype=np.float32)[0])

    idx_bs = idx.T  # [B, S]

    WihT = np.ascontiguousarray(W_ih.T)
    WhhT = np.ascontiguousarray(W_hh.T)
    Wbil = np.ascontiguousarray(W_bil)
    WptrT = np.ascontiguousarray(W_ptr.T)
    bih = b_ih[None, :]
    bhh = b_hh[None, :]

    in_maps = []
    plans = []
    for c in range(NCORES):
        bs = slice(c * BC, (c + 1) * BC)
        vs = slice(c * VC, (c + 1) * VC)
        Km, Bm, Om, chunk_groups, NL = build_lane_plan(idx_bs, c)
        plans.append((chunk_groups, NL))
        in_maps.append({
            "embT": np.ascontiguousarray(emb[bs].T),
            "hidT": np.ascontiguousarray(hidf[bs].T),
            "hid": np.ascontiguousarray(hidf[bs]),
            "WihT": WihT, "WhhT": WhhT, "bih": bih, "bhh": bhh,
            "Wbil": Wbil, "WptrT": WptrT,
            "encT": np.ascontiguousarray(enc[:, bs, :].transpose(1, 2, 0)),
            "WoutT": np.ascontiguousarray(W_out[vs].T).astype(ml_dtypes.bfloat16),
            "bout": b_out[None, vs].astype(ml_dtypes.bfloat16),
            "Kmat": Km, "BMT": Bm, "OHm": Om,
        })
    # all cores must share one NEFF: unify lane-plan shapes to the max
    gmax = [max(p[0][ci] for p in plans) for ci in range(len(CHUNKS))]
    NLu = 128 * sum(gmax)
    for c in range(NCORES):
        chunk_groups, NL = plans[c]
        Kn = np.zeros((512, NLu), ml_dtypes.bfloat16)
        Bn = np.zeros((NLu, 128), ml_dtypes.bfloat16)
        On = np.zeros((NLu, N_CHUNK), ml_dtypes.bfloat16)
        src_lo = 0
        dst_lo = 0
        for ci in range(len(CHUNKS)):
            n = 128 * chunk_groups[ci]
            Kn[:, dst_lo:dst_lo + n] = in_maps[c]["Kmat"][:, src_lo:src_lo + n]
            Bn[dst_lo:dst_lo + n] = in_maps[c]["BMT"][src_lo:src_lo + n]
            On[dst_lo:dst_lo + n] = in_maps[c]["OHm"][src_lo:src_lo + n]
            src_lo += n
            dst_lo += 128 * gmax[ci]
        in_maps[c]["Kmat"] = Kn
        in_maps[c]["BMT"] = Bn
        in_maps[c]["OHm"] = On
    return in_maps, b_ptr, gmax, NLu


def kernel(**inputs):
    global LAST_RESULT
    in_maps, b_ptr, gmax, NLu = make_in_maps(inputs)
    nc = _get_nc(NLu, gmax, b_ptr)
    res = run_bass_kernel_spmd(nc, in_maps, core_ids=list(range(NCORES)),
                               trace=TRACE)
    LAST_RESULT = res
    results = res.results

    out = np.empty((B, V), np.float32)
    h_new = np.empty((B, H), np.float32)
    enc_attn = np.empty((B, S), np.float32)
    prob_ptr = np.empty((B, 1), np.float32)
    for c in range(NCORES):
        r = results[c]
        out[:, c * VC:(c + 1) * VC] = r["out_flat"].reshape(128, VC)
        h_new[c * BC:(c + 1) * BC] = r["hnew_o"]
        enc_attn[c * BC:(c + 1) * BC] = r["attn_o"]
        prob_ptr[c * BC:(c + 1) * BC] = r["pptr_o"]
    return out, h_new[None], enc_attn[:, :, None], prob_ptr
